# Initial kernel scaffold
#
"""Trainium2 Bass kernel for nn_MeanAggregator:

    out = features[nodes] + mean(features[neigh_idx], axis=1)

Sharding: batch data-parallel across 8 NeuronCores (12500 nodes/core,
padded to 12544 = 98 tiles of 128), feature table replicated per core.
Per 128-node tile one SWDGE indirect DMA gathers 17*128 rows (self + 16
neighbors, 512B each) into SBUF [128, 17*128] (partition p = node, free
block t = gathered row t). DVE tree-adds the 16 neighbor blocks, then a
fused scalar_tensor_tensor computes (sum * 1/16) + self. HWDGE writes
the [128,128] result tile back to DRAM. No cross-core communication.
"""

import numpy as np

import concourse.bass as bass
import concourse.mybir as mybir
import concourse.tile as tile
from concourse import bacc
from concourse.bass_utils import run_bass_kernel_spmd

NUM_NODES = 1_000_000
FEAT = 128
BATCH = 100_000
K = 16
BLK = K + 1  # rows gathered per node: self + K neighbors
CORES = 8
B_CORE = BATCH // CORES  # 12500
P = 128
N_TILES = (B_CORE + P - 1) // P  # 98
B_PAD = N_TILES * P  # 12544

_NC_CACHE: dict = {}


def build_nc(n_tiles=N_TILES, table_rows=NUM_NODES, gbufs=4, abufs=3, obufs=3):
    f32 = mybir.dt.float32
    i32 = mybir.dt.int32
    nc = bacc.Bacc(None, debug=False)
    feat_t = nc.dram_tensor("features", [table_rows, FEAT], f32, kind="ExternalInput")
    idx_t = nc.dram_tensor("idx", [P, n_tiles * BLK], i32, kind="ExternalInput")
    out_t = nc.dram_tensor("out", [n_tiles, P, FEAT], f32, kind="ExternalOutput")

    with tile.TileContext(nc) as tc:
        with (
            tc.tile_pool(name="idxp", bufs=1) as idxp,
            tc.tile_pool(name="gp", bufs=gbufs) as gp,
            tc.tile_pool(name="tp", bufs=abufs) as tp,
            tc.tile_pool(name="op", bufs=obufs) as op,
        ):
            idx_sb = idxp.tile([P, n_tiles * BLK], i32)
            nc.sync.dma_start(out=idx_sb[:], in_=idx_t[:])
            for n in range(n_tiles):
                # G[p, t*128:(t+1)*128] = features[idx[p, n*17+t]]
                G = gp.tile([P, BLK * FEAT], f32)
                nc.gpsimd.indirect_dma_start(
                    out=G[:],
                    out_offset=None,
                    in_=feat_t[:],
                    in_offset=bass.IndirectOffsetOnAxis(
                        ap=idx_sb[:, n * BLK : (n + 1) * BLK], axis=0
                    ),
                )
                # tree-add the 16 neighbor blocks (cols 128..2176)
                A = tp.tile([P, 1920], f32)
                nc.vector.tensor_add(A[:, 0:1024], G[:, 128:1152], G[:, 1152:2176])
                nc.vector.tensor_add(A[:, 1024:1536], A[:, 0:512], A[:, 512:1024])
                nc.vector.tensor_add(A[:, 1536:1792], A[:, 1024:1280], A[:, 1280:1536])
                nc.vector.tensor_add(A[:, 1792:1920], A[:, 1536:1664], A[:, 1664:1792])
                O = op.tile([P, FEAT], f32)
                nc.vector.scalar_tensor_tensor(
                    out=O[:],
                    in0=A[:, 1792:1920],
                    scalar=1.0 / K,
                    in1=G[:, 0:FEAT],
                    op0=mybir.AluOpType.mult,
                    op1=mybir.AluOpType.add,
                )
                nc.sync.dma_start(out=out_t[n], in_=O[:])
    nc.finalize()
    return nc


def _get_nc():
    if "nc" not in _NC_CACHE:
        _NC_CACHE["nc"] = build_nc()
    return _NC_CACHE["nc"]


def _shard_idx(idx_all):
    """idx_all [BATCH, BLK] int32 -> per-core [P, N_TILES*BLK] tiles-major layout."""
    maps = []
    for c in range(CORES):
        sh = idx_all[c * B_CORE : (c + 1) * B_CORE]
        pad = np.zeros((B_PAD, BLK), np.int32)
        pad[:B_CORE] = sh
        r = pad.reshape(N_TILES, P, BLK).transpose(1, 0, 2).reshape(P, N_TILES * BLK)
        maps.append(np.ascontiguousarray(r))
    return maps


def run_sharded(features, nodes, neigh_idx, trace=False, **spmd_kwargs):
    features = np.ascontiguousarray(np.asarray(features, dtype=np.float32))
    nodes = np.asarray(nodes).astype(np.int32)
    neigh_idx = np.asarray(neigh_idx).astype(np.int32)
    idx_all = np.concatenate([nodes[:, None], neigh_idx], axis=1)
    in_maps = [
        {"features": features, "idx": idx_c} for idx_c in _shard_idx(idx_all)
    ]
    res = run_bass_kernel_spmd(
        _get_nc(), in_maps, list(range(CORES)), trace=trace, **spmd_kwargs
    )
    out = np.concatenate(
        [res.results[c]["out"].reshape(B_PAD, FEAT)[:B_CORE] for c in range(CORES)],
        axis=0,
    )
    return out, res


def kernel(**inputs):
    num_sample = int(np.asarray(inputs["num_sample"]))
    assert num_sample == K, f"kernel hardcodes K={K}, got {num_sample}"
    out, _ = run_sharded(
        inputs["features"], inputs["nodes"], inputs["neigh_idx"], trace=False
    )
    return out


# revision 2
# speedup vs baseline: 6.3232x; 6.3232x over previous
"""Trainium2 Bass kernel for nn_MeanAggregator:

    out = features[nodes] + mean(features[neigh_idx], axis=1)

Sharding: batch data-parallel across 8 NeuronCores (12500 nodes/core,
padded to 12544 = 98 tiles of 128), feature table replicated per core.
Per 128-node tile one SWDGE indirect DMA gathers 17*128 rows (self + 16
neighbors, 512B each) into SBUF [128, 17*128] (partition p = node, free
block t = gathered row t). DVE tree-adds the 16 neighbor blocks, then a
fused scalar_tensor_tensor computes (sum * 1/16) + self. HWDGE writes
the [128,128] result tile back to DRAM. No cross-core communication.
"""

import numpy as np

import concourse.bass as bass
import concourse.mybir as mybir
import concourse.tile as tile
from concourse import bacc
from concourse.bass_utils import run_bass_kernel_spmd

NUM_NODES = 1_000_000
FEAT = 128
BATCH = 100_000
K = 16
BLK = K + 1  # rows gathered per node: self + K neighbors
CORES = 8
B_CORE = BATCH // CORES  # 12500
P = 128
N_TILES = (B_CORE + P - 1) // P  # 98
B_PAD = N_TILES * P  # 12544

_NC_CACHE: dict = {}


def build_nc(n_tiles=N_TILES, table_rows=NUM_NODES, gbufs=4, abufs=3, obufs=3):
    f32 = mybir.dt.float32
    i32 = mybir.dt.int32
    nc = bacc.Bacc(None, debug=False)
    feat_t = nc.dram_tensor("features", [table_rows, FEAT], f32, kind="ExternalInput")
    idx_t = nc.dram_tensor("idx", [P, n_tiles * BLK], i32, kind="ExternalInput")
    out_t = nc.dram_tensor("out", [n_tiles, P, FEAT], f32, kind="ExternalOutput")

    with tile.TileContext(nc) as tc:
        with (
            tc.tile_pool(name="idxp", bufs=1) as idxp,
            tc.tile_pool(name="gp", bufs=gbufs) as gp,
            tc.tile_pool(name="tp", bufs=abufs) as tp,
            tc.tile_pool(name="op", bufs=obufs) as op,
        ):
            # Pad each 512B gather block to 640B so per-partition blocks do
            # NOT coalesce into one descriptor: the HW DGE consumes one
            # offset per descriptor, streaming the descriptor's extent from
            # features[offset]. Contiguous blocks coalesced -> only the
            # first offset per partition was honored.
            STRIDE = 160  # f32 elems per block slot (128 data + 32 pad)
            idx_sb = idxp.tile([P, n_tiles * BLK], i32)
            nc.sync.dma_start(out=idx_sb[:], in_=idx_t[:])
            for n in range(n_tiles):
                # G[p, t, 0:128] = features[idx[p, n*17+t]]
                G = gp.tile([P, BLK, STRIDE], f32)
                nc.gpsimd.indirect_dma_start(
                    out=G[:, :, 0:FEAT],
                    out_offset=None,
                    in_=feat_t[:],
                    in_offset=bass.IndirectOffsetOnAxis(
                        ap=idx_sb[:, n * BLK : (n + 1) * BLK], axis=0
                    ),
                )
                # tree-add the 16 neighbor blocks (t=1..16)
                A = tp.tile([P, 15, FEAT], f32)
                nc.vector.tensor_add(
                    A[:, 0:8, :], G[:, 1:9, 0:FEAT], G[:, 9:17, 0:FEAT]
                )
                nc.vector.tensor_add(A[:, 8:12, :], A[:, 0:4, :], A[:, 4:8, :])
                nc.vector.tensor_add(A[:, 12:14, :], A[:, 8:10, :], A[:, 10:12, :])
                nc.vector.tensor_add(A[:, 14:15, :], A[:, 12:13, :], A[:, 13:14, :])
                O = op.tile([P, FEAT], f32)
                nc.vector.scalar_tensor_tensor(
                    out=O[:],
                    in0=A[:, 14, :],
                    scalar=1.0 / K,
                    in1=G[:, 0, 0:FEAT],
                    op0=mybir.AluOpType.mult,
                    op1=mybir.AluOpType.add,
                )
                nc.sync.dma_start(out=out_t[n], in_=O[:])
    nc.finalize()
    return nc


def _get_nc():
    if "nc" not in _NC_CACHE:
        _NC_CACHE["nc"] = build_nc()
    return _NC_CACHE["nc"]


def _shard_idx(idx_all):
    """idx_all [BATCH, BLK] int32 -> per-core [P, N_TILES*BLK] tiles-major layout."""
    maps = []
    for c in range(CORES):
        sh = idx_all[c * B_CORE : (c + 1) * B_CORE]
        pad = np.zeros((B_PAD, BLK), np.int32)
        pad[:B_CORE] = sh
        r = pad.reshape(N_TILES, P, BLK).transpose(1, 0, 2).reshape(P, N_TILES * BLK)
        maps.append(np.ascontiguousarray(r))
    return maps


def run_sharded(features, nodes, neigh_idx, trace=False, **spmd_kwargs):
    features = np.ascontiguousarray(np.asarray(features, dtype=np.float32))
    nodes = np.asarray(nodes).astype(np.int32)
    neigh_idx = np.asarray(neigh_idx).astype(np.int32)
    idx_all = np.concatenate([nodes[:, None], neigh_idx], axis=1)
    in_maps = [
        {"features": features, "idx": idx_c} for idx_c in _shard_idx(idx_all)
    ]
    res = run_bass_kernel_spmd(
        _get_nc(), in_maps, list(range(CORES)), trace=trace, **spmd_kwargs
    )
    out = np.concatenate(
        [res.results[c]["out"].reshape(B_PAD, FEAT)[:B_CORE] for c in range(CORES)],
        axis=0,
    )
    return out, res


def kernel(**inputs):
    num_sample = int(np.asarray(inputs["num_sample"]))
    assert num_sample == K, f"kernel hardcodes K={K}, got {num_sample}"
    out, _ = run_sharded(
        inputs["features"], inputs["nodes"], inputs["neigh_idx"], trace=False
    )
    return out
